# revision 95
# baseline (speedup 1.0000x reference)
"""Trainium2 Bass kernel for nn_EncoderOnlyBlock (4-head full-dim encoder block).

Sharding: fully data-parallel, no collectives. 8 cores = (batch b, seq-half).
Each core computes its 1024 query tokens end-to-end for all 4 heads.

Algebraic folds (host, fp32):
  scores[i,j] = (x_i Wq + bq)·(x_j Wk + bk) / 32
              = (x_i G + w~)·x_j / 32  + terms constant over j (softmax-invariant)
    with G = Wq Wk^T, w~ = Wk bq.  K projection eliminated entirely.
  head_h @ W1_h = A_h @ (x F_h),  F_h = Wv_h W1_h;  bv/b1 fold into
    cvec = b1 + sum_h bv_h W1_h which is folded into xres on host.
  Softmax normalization deferred: proj_unnorm = sum_j exp(s) V', scaled by
    rec = 1/rowsum at PSUM->SBUF accumulation ([P,1] per-partition scalar).

fp8 (e4m3, DoubleRow perf mode = 2 K-tiles per pass) for all attention
matmuls: T' = x G'8, scores = T'8 @ x8, V' = x8 @ F'8, proj = expS8 @ V'8.
Host scales G'=32G, F'=64F keep fp8 operands in the normal range; the 32
folds into the exp scale (1/1024), the 64 into the u1 residual STT (1/64).
W2/LN tail stays bf16/f32 exactly as the bf16 baseline.

Per-core math:
  x_perm = [own-half tokens; other-half tokens]            (host permute)
  V'_h  = x_perm @ F'_h                     [sj, d]  fp8 out
  T'_h  = G'_h^T @ x_perm^T[:, :1024] + w~' [e, si]  fp8 out
  S     = T'^T x^T / 1024; E = exp(S) (fp8), r = rowsum(E) (f32 accum)
  proj += (E_h @ V'_h) * (1/r_h)            [si, d]  f32 accum in SBUF
  u1    = xres' + proj/64   (xres' = x_own + cvec, host)
  LN1 -> y; u2 = y + y@W2 + b2; LN2 -> out   (bf16 matmul, f32 LN)
"""

import numpy as np
import ml_dtypes

BF = ml_dtypes.bfloat16
F8 = ml_dtypes.float8_e4m3
P = 128
D = 1024
S = 2048
SI = 1024
H = 4
ET = D // P       # 8 e/d/f 128-blocks
SJT = S // P      # 16 sj 128-blocks
SIT = SI // P     # 8 si 128-blocks
EPS = 1e-5

_CACHE = {}


def _emit(nc, tc, A, trivial_gbe):
    """Emit the per-core program. A: dict name -> dram AP."""
    from contextlib import ExitStack

    import concourse.bass as bass
    import concourse.mybir as mybir
    from concourse.masks import make_identity

    f32 = mybir.dt.float32
    bf16 = mybir.dt.bfloat16
    f8 = mybir.dt.float8e4
    Act = mybir.ActivationFunctionType
    Alu = mybir.AluOpType
    DR = mybir.MatmulPerfMode.DoubleRow

    with ExitStack() as ctx:
        consts = ctx.enter_context(tc.tile_pool(name="consts", bufs=1))
        psA = ctx.enter_context(tc.tile_pool(name="psA", bufs=3, space="PSUM"))
        psB = ctx.enter_context(tc.tile_pool(name="psB", bufs=2, space="PSUM"))

        identb = consts.tile([P, P], bf16, tag="identb")
        make_identity(nc, identb[:])
        wtr_sb = consts.tile([P, H * ET], f32, tag="wtr")
        nc.sync.dma_start(out=wtr_sb[:], in_=A["wtr"][:])
        buv_sb = consts.tile([1, D], bf16, tag="buv")
        nc.sync.dma_start(out=buv_sb[:], in_=A["buv"][:])
        ones_sb = consts.tile([1, P], bf16, tag="ones")
        nc.vector.memset(ones_sb[:], 1.0)
        ones8 = consts.tile([P, 2, P], f8, tag="ones8")
        nc.vector.memset(ones8[:], 1.0)
        eps_sb = consts.tile([P, 1], f32, tag="eps")
        nc.vector.memset(eps_sb[:], EPS)
        nl16_sb = consts.tile([P, 1], f32, tag="nl16")
        nc.vector.memset(nl16_sb[:], -2.772588722239781)

        head_ctx = ExitStack()
        # LN1 pools live in the outer ctx: stage A (u1 -> LN1 -> y) is emitted
        # inline as head 3's proj tiles complete, so the LN phase starts hot
        lnp = ctx.enter_context(tc.tile_pool(name="lnp", bufs=1))
        u_pool = ctx.enter_context(tc.tile_pool(name="up", bufs=2))
        sq_pool = ctx.enter_context(tc.tile_pool(name="sq", bufs=2))
        st_pool = ctx.enter_context(tc.tile_pool(name="st", bufs=8))
        y_sb = lnp.tile([P, SIT, D], bf16, tag="y")

        xpool = head_ctx.enter_context(tc.tile_pool(name="xp", bufs=1))
        f_pool = head_ctx.enter_context(tc.tile_pool(name="fp8", bufs=2))
        g_pool = head_ctx.enter_context(tc.tile_pool(name="gp8", bufs=2))
        mu_pool = head_ctx.enter_context(tc.tile_pool(name="mu", bufs=2))
        tt_pool = head_ctx.enter_context(tc.tile_pool(name="tt", bufs=2))
        e_pool = head_ctx.enter_context(tc.tile_pool(name="es", bufs=1))
        proj_pool = head_ctx.enter_context(tc.tile_pool(name="pj", bufs=1))
        red_pool = head_ctx.enter_context(tc.tile_pool(name="red", bufs=1))
        rec_pool = head_ctx.enter_context(tc.tile_pool(name="rec", bufs=2))

        xq8_sb = xpool.tile([P, ET, SI], f8, tag="xq8")
        for c2 in range(4):
            nc.sync.dma_start(
                out=xq8_sb[:, 2 * c2:2 * c2 + 2, :],
                in_=A["xq8"][2 * c2 * P:(2 * c2 + 2) * P, :].rearrange(
                    "(c p) s -> p c s", p=P),
            )
        x8_sb = xpool.tile([P, ET, S], f8, tag="x8")
        for c2 in range(4):
            nc.gpsimd.dma_start(
                out=x8_sb[:, 2 * c2:2 * c2 + 2, :],
                in_=A["x8"][2 * c2 * P:(2 * c2 + 2) * P, :].rearrange(
                    "(c p) s -> p c s", p=P),
            )
        xn8_sb = xpool.tile([P, SJT, D], f8, tag="xn8")
        for j4 in range(4):
            nc.gpsimd.dma_start(
                out=xn8_sb[:, 4 * j4:4 * j4 + 4, :],
                in_=A["xn8"][4 * j4 * P:(4 * j4 + 4) * P, :].rearrange(
                    "(j p) d -> p j d", p=P),
            )

        # proj accumulator preloaded with 64*(x_own + cvec) in f32: the
        # residual + attention bias ride along exactly; u1 = proj/64.
        # (Init DMAs are emitted inside head 0 so they don't contend with the
        # startup-critical xq8/g8/x8 loads for HBM bandwidth.)
        proj_sb = proj_pool.tile([P, SIT, D], f32, tag="proj")

        if not trivial_gbe:
            gbe_sb = lnp.tile([P, 4, D], f32, tag="gbe")
            gbe_bc = bass.AP(
                tensor=A["gbe"].tensor, offset=A["gbe"].offset,
                ap=[[0, P], A["gbe"].ap[0], A["gbe"].ap[1]],
            )
            nc.gpsimd.dma_start(out=gbe_sb[:], in_=gbe_bc)

        def ln_stats(src, rsum):
            """-> (mu, rstd) [P,1] tiles from src [P,D] + its row-sum."""
            sq = sq_pool.tile([P, D], f32, tag="sq")
            sumsq = st_pool.tile([P, 1], f32, tag="sumsq")
            nc.scalar.activation(out=sq[:], in_=src, func=Act.Square,
                                 accum_out=sumsq[:])
            mu = st_pool.tile([P, 1], f32, tag="mu")
            nc.scalar.mul(mu[:], rsum, 1.0 / D)
            # (rsum*mu - sumsq) = -D*var;  std = sqrt(-1/D * that + eps)
            nv = st_pool.tile([P, 1], f32, tag="nv")
            nc.vector.scalar_tensor_tensor(
                out=nv[:], in0=rsum, scalar=mu[:], in1=sumsq[:],
                op0=Alu.mult, op1=Alu.subtract,
            )
            rstd = st_pool.tile([P, 1], f32, tag="rstd")
            nc.scalar.activation(out=rstd[:], in_=nv[:], func=Act.Sqrt,
                                 scale=-1.0 / D, bias=eps_sb[:])
            nc.vector.reciprocal(rstd[:], rstd[:])
            return mu, rstd

        def stage_A(t):
            """u1 = proj/64 -> LN1 -> y_sb[:, t]."""
            u1 = u_pool.tile([P, D], f32, tag="u")
            rs1 = st_pool.tile([P, 1], f32, tag="rs")
            nc.scalar.activation(
                out=u1[:], in_=proj_sb[:, t, :], func=Act.Copy,
                scale=1.0 / 64.0, accum_out=rs1[:],
            )
            mu1, rstd1 = ln_stats(u1[:], rs1[:])
            yt_t = y_sb[:, t, :]
            nc.vector.tensor_scalar(
                yt_t, u1[:], scalar1=mu1[:], scalar2=rstd1[:],
                op0=Alu.subtract, op1=Alu.mult,
            )
            if not trivial_gbe:
                nc.gpsimd.tensor_mul(yt_t, yt_t, gbe_sb[:, 0, :])
                nc.gpsimd.tensor_add(yt_t, yt_t, gbe_sb[:, 1, :])

        for h in range(H):
            # ---- T'^T = G'^T @ x^T[:, :1024] + w~' : [e, si]
            dmae = nc.sync if h == 0 else nc.scalar
            f8_sb = f_pool.tile([P, ET, D], f8, tag="f8")
            for k2 in range(2):
                dmae.dma_start(
                    out=f8_sb[:, 4 * k2:4 * k2 + 4, :],
                    in_=A["f8"][h, 4 * k2 * P:(4 * k2 + 4) * P, :].rearrange(
                        "(k p) d -> p k d", p=P),
                )
            g8_sb = g_pool.tile([P, ET, ET, P], f8, tag="g8")
            for c2 in range(2):
                dmae.dma_start(
                    out=g8_sb[:, 4 * c2:4 * c2 + 4], in_=A["g8"][h, :, 4 * c2:4 * c2 + 4]
                )
            tt_sb = tt_pool.tile([P, ET, SI], f8, tag="tt")
            for c in range(ET):
                ps = psA.tile([P, D], f32, tag="psA")
                for nb in range(2):
                    for kp in range(ET // 2):
                        nc.tensor.matmul(
                            ps[:, nb * 512:(nb + 1) * 512],
                            lhsT=g8_sb[:, c, 2 * kp:2 * kp + 2, :],
                            rhs=xq8_sb[:, 2 * kp:2 * kp + 2, nb * 512:(nb + 1) * 512],
                            start=(kp == 0), stop=(kp == ET // 2 - 1),
                            perf_mode=DR,
                        )
                nc.scalar.activation(
                    out=tt_sb[:, c, :], in_=ps[:], func=Act.Identity,
                    bias=wtr_sb[:, h * ET + c:h * ET + c + 1],
                )
            if h == 0:
                for t2 in range(4):
                    nc.scalar.dma_start(
                        out=proj_sb[:, 2 * t2:2 * t2 + 2, :],
                        in_=A["xres"][2 * t2 * P:(2 * t2 + 2) * P, :].rearrange(
                            "(t p) d -> p t d", p=P),
                    )

            # ---- scores^T per key-block: S^T[j, i] = x_j . T'_i, exp'd straight
            # to fp8 (feeds proj as lhsT with no transposes). Row sums via tiny
            # ones-DoubleRow matmuls accumulated over all 16 key blocks.
            expS_sb = e_pool.tile([P, SJT, SI], f8, tag="expS")
            for jblk in range(SJT):
                ps = psA.tile([P, SI], f32, tag="psA")
                for nb in range(2):
                    for kp in range(ET // 2):
                        nc.tensor.matmul(
                            ps[:, nb * 512:(nb + 1) * 512],
                            lhsT=x8_sb[:, 2 * kp:2 * kp + 2, jblk * P:(jblk + 1) * P],
                            rhs=tt_sb[:, 2 * kp:2 * kp + 2, nb * 512:(nb + 1) * 512],
                            start=(kp == 0), stop=(kp == ET // 2 - 1),
                            perf_mode=DR,
                        )
                # exp/16: keeps M = expS @ x inside fp8 range; the 16 cancels
                # against the rowsum in the final rec normalization
                nc.scalar.activation(
                    out=expS_sb[:, jblk, :], in_=ps[:],
                    func=Act.Exp, scale=1.0 / 1024.0, bias=nl16_sb[:],
                )
            # rowsum broadcast to all 128 partitions via all-ones lhsT
            rps = psA.tile([P, SI], f32, tag="psA")
            for jp in range(SJT // 2):
                for nb in range(2):
                    nc.tensor.matmul(
                        rps[:, nb * 512:(nb + 1) * 512],
                        lhsT=ones8[:],
                        rhs=expS_sb[:, 2 * jp:2 * jp + 2, nb * 512:(nb + 1) * 512],
                        start=(jp == 0), stop=(jp == SJT // 2 - 1),
                        perf_mode=DR,
                    )
            # 1/rowsum, regrouped to [P si-in-tile, tile] via scatter DMA
            r_sb = red_pool.tile([P, SI], f32, tag="rrow")
            nc.vector.tensor_copy(r_sb[:], rps[:])
            nc.sync.dma_start(out=A["rsc"][h], in_=r_sb[0:1, :])
            rec_sb = rec_pool.tile([P, SIT], f32, tag="rec")
            nc.sync.dma_start(
                out=rec_sb[:], in_=A["rsc"][h].rearrange("(t p) -> p t", p=P)
            )
            nc.vector.reciprocal(rec_sb[:], rec_sb[:])

            # ---- M^T = x^T @ expS^T : [d, si]  (the (A@x) reassociation —
            # contracts the 2048-key dim before touching F)
            mu8 = mu_pool.tile([P, ET, SI], f8, tag="mu8")
            for dblk in range(ET):
                ps = psA.tile([P, SI], f32, tag="psA")
                for nb in range(2):
                    for jp in range(SJT // 2):
                        nc.tensor.matmul(
                            ps[:, nb * 512:(nb + 1) * 512],
                            lhsT=xn8_sb[:, 2 * jp:2 * jp + 2,
                                        dblk * P:(dblk + 1) * P],
                            rhs=expS_sb[:, 2 * jp:2 * jp + 2,
                                        nb * 512:(nb + 1) * 512],
                            start=(jp == 0), stop=(jp == SJT // 2 - 1),
                            perf_mode=DR,
                        )
                nc.scalar.copy(mu8[:, dblk, :], ps[:])

            # ---- proj += (M @ F'_h) * rec_h
            for t in range(SIT):
                ps = psA.tile([P, D], f32, tag="psA")
                for nb in range(2):
                    for kp in range(ET // 2):
                        nc.tensor.matmul(
                            ps[:, nb * 512:(nb + 1) * 512],
                            lhsT=mu8[:, 2 * kp:2 * kp + 2, t * P:(t + 1) * P],
                            rhs=f8_sb[:, 2 * kp:2 * kp + 2, nb * 512:(nb + 1) * 512],
                            start=(kp == 0), stop=(kp == ET // 2 - 1),
                            perf_mode=DR,
                        )
                nc.vector.scalar_tensor_tensor(
                    out=proj_sb[:, t, :], in0=ps[:], scalar=rec_sb[:, t:t + 1],
                    in1=proj_sb[:, t, :], op0=Alu.mult, op1=Alu.add,
                )
                if h == H - 1:
                    stage_A(t)

        head_ctx.close()

        # ================= yT -> FFN2 -> LN2 (y already computed inline) ======
        with ExitStack() as lctx:
            yt_pool = lctx.enter_context(tc.tile_pool(name="yt", bufs=3))
            w2_pool = lctx.enter_context(tc.tile_pool(name="w2", bufs=1))
            ot_pool = lctx.enter_context(tc.tile_pool(name="ot", bufs=3))

            w28_sb = w2_pool.tile([P, ET, D], bf16, tag="w2")
            nc.sync.dma_start(
                out=w28_sb[:], in_=A["w2"][:].rearrange("(k p) d -> p k d", p=P)
            )

            yt_tiles = []
            for t in range(SIT):
                # transpose this tile's 8 f-blocks -> yT columns for its z-chain
                yt_t = y_sb[:, t, :]
                yt_tile = yt_pool.tile([P, ET, P], bf16, tag="yt", name=f"yt{t}")
                pb = psB.tile([P, 1024], bf16, tag="psB")
                for fb in range(ET):
                    nc.tensor.transpose(
                        pb[:, fb * P:(fb + 1) * P], yt_t[:, fb * P:(fb + 1) * P],
                        identb[:],
                    )
                nc.vector.tensor_copy(
                    yt_tile[:], pb[:].rearrange("p (f c) -> p f c", c=P)
                )
                yt_tiles.append(yt_tile)

            # Stage B (all tiles): z-chain -> u2 -> LN2 -> out
            for t in range(SIT):
                ps = psA.tile([P, 1024], f32, tag="psA")
                for nb in range(2):
                    for kc in range(ET):
                        nc.tensor.matmul(
                            ps[:, nb * 512:(nb + 1) * 512],
                            lhsT=yt_tiles[t][:, kc, :],
                            rhs=w28_sb[:, kc, nb * 512:(nb + 1) * 512],
                            start=(kc == 0), stop=False,
                        )
                    nc.tensor.matmul(
                        ps[:, nb * 512:(nb + 1) * 512],
                        lhsT=ones_sb[:, :],
                        rhs=buv_sb[:, nb * 512:(nb + 1) * 512],
                        start=False, stop=True,
                    )
                u2 = u_pool.tile([P, D], f32, tag="u")
                rs2 = st_pool.tile([P, 1], f32, tag="rs")
                nc.vector.scalar_tensor_tensor(
                    out=u2[:], in0=ps[:], scalar=1.0,
                    in1=y_sb[:, t, :], op0=Alu.mult, op1=Alu.add,
                    accum_out=rs2[:],
                )
                mu2, rstd2 = ln_stats(u2[:], rs2[:])
                ot = ot_pool.tile([P, D], f32, tag="ot")
                nc.vector.tensor_scalar(
                    ot[:], u2[:], scalar1=mu2[:], scalar2=rstd2[:],
                    op0=Alu.subtract, op1=Alu.mult,
                )
                if not trivial_gbe:
                    nc.gpsimd.tensor_mul(ot[:], ot[:], gbe_sb[:, 2, :])
                    nc.gpsimd.tensor_add(ot[:], ot[:], gbe_sb[:, 3, :])
                eng = nc.sync if t % 2 == 0 else nc.gpsimd
                eng.dma_start(out=A["out"][t * P:(t + 1) * P, :], in_=ot[:])


def _build(trivial_gbe):
    import concourse.bass as bass
    import concourse.mybir as mybir
    import concourse.tile as tile
    from concourse import bacc

    f32 = mybir.dt.float32
    bf16 = mybir.dt.bfloat16
    f8 = mybir.dt.float8e4

    nc = bacc.Bacc("TRN2", target_bir_lowering=False, debug=False, num_devices=8)
    A = {}

    def din(name, shape, dt):
        A[name] = nc.dram_tensor(name, shape, dt, kind="ExternalInput").ap()

    din("x8", [D, S], f8)
    din("xn8", [S, D], f8)
    din("xq8", [D, SI], f8)
    din("xres", [SI, D], f32)
    din("g8", [H, P, ET, ET, P], f8)
    din("f8", [H, D, D], f8)
    din("wtr", [P, H * ET], f32)
    din("w2", [D, D], bf16)
    din("buv", [1, D], bf16)
    if not trivial_gbe:
        din("gbe", [4, D], f32)
    A["rsc"] = nc.dram_tensor("rsc", [H, SI], f32, kind="Internal").ap()
    A["out"] = nc.dram_tensor("out", [SI, D], f32, kind="ExternalOutput").ap()

    with tile.TileContext(nc) as tc:
        _emit(nc, tc, A, trivial_gbe)
    nc.compile()
    return nc


def _get_nc(trivial_gbe=True):
    key = ("nc", trivial_gbe)
    if key not in _CACHE:
        _CACHE[key] = _build(trivial_gbe)
    return _CACHE[key]


def _prep_inputs(inputs):
    x = np.ascontiguousarray(inputs["embedding_matrix"], dtype=np.float32)
    Wq = np.asarray(inputs["Wq"], np.float32)
    bq = np.asarray(inputs["bq"], np.float32)
    Wv = np.asarray(inputs["Wv"], np.float32)
    bv = np.asarray(inputs["bv"], np.float32)
    Wk = np.asarray(inputs["Wk"], np.float32)
    W1 = np.asarray(inputs["W1"], np.float32)
    b1 = np.asarray(inputs["b1"], np.float32)
    W2 = np.asarray(inputs["W2"], np.float32)
    b2 = np.asarray(inputs["b2"], np.float32)
    g1 = np.asarray(inputs["g1"], np.float32)
    be1 = np.asarray(inputs["be1"], np.float32)
    g2 = np.asarray(inputs["g2"], np.float32)
    be2 = np.asarray(inputs["be2"], np.float32)

    trivial = (
        np.array_equal(g1, np.ones(D, np.float32))
        and np.array_equal(g2, np.ones(D, np.float32))
        and np.array_equal(be1, np.zeros(D, np.float32))
        and np.array_equal(be2, np.zeros(D, np.float32))
    )

    # host folds
    G = np.stack([32.0 * (Wq[h] @ Wk[h].T) for h in range(H)])        # [H, D, D]
    F = np.stack([64.0 * (Wv[h] @ W1[h * D:(h + 1) * D]) for h in range(H)])
    wt = np.stack([32.0 * (Wk[h] @ bq[h]) for h in range(H)])          # [H, D]
    cvec = b1 + sum(bv[h] @ W1[h * D:(h + 1) * D] for h in range(H))   # [D]

    def pack_w(W, dtyp):  # [H, D, D] -> [H, P(in-blk), c, kc, P] lhsT blocks
        return np.ascontiguousarray(
            W.reshape(H, ET, P, ET, P).transpose(0, 2, 3, 1, 4).astype(dtyp)
        )

    g8 = pack_w(G, F8)
    f8 = np.ascontiguousarray(F.astype(F8))
    wtr = np.ascontiguousarray(wt.reshape(H, ET, P).transpose(2, 0, 1).reshape(P, H * ET))
    w2b = np.ascontiguousarray(W2.astype(BF))
    buv = np.ascontiguousarray(b2.reshape(1, D).astype(BF))

    shared = {"g8": g8, "f8": f8, "wtr": wtr, "w2": w2b, "buv": buv}
    if not trivial:
        shared["gbe"] = np.ascontiguousarray(np.stack([g1, be1, g2, be2]))
    in_maps = []
    xT8 = [np.ascontiguousarray(x[b].T.astype(F8)) for b in range(4)]
    xN8 = [np.ascontiguousarray(x[b].astype(F8)) for b in range(4)]
    for core in range(8):
        b, half = core // 2, core % 2
        own = x[b, half * SI:(half + 1) * SI]
        m = dict(shared)
        m["x8"] = xT8[b]
        m["xn8"] = xN8[b]
        m["xq8"] = np.ascontiguousarray(xT8[b][:, half * SI:(half + 1) * SI])
        m["xres"] = np.ascontiguousarray(64.0 * (own + cvec[None, :]))
        in_maps.append(m)
    return trivial, in_maps


def kernel(**inputs):
    from concourse.bass_utils import run_bass_kernel_spmd

    trivial, in_maps = _prep_inputs(inputs)
    nc = _get_nc(trivial)
    res = run_bass_kernel_spmd(nc, in_maps, core_ids=list(range(8)))
    out = np.empty((4, S, D), np.float32)
    for core in range(8):
        b, half = core // 2, core % 2
        out[b, half * SI:(half + 1) * SI] = res.results[core]["out"]
    return out


# revision 98
# speedup vs baseline: 1.0183x; 1.0183x over previous
"""Trainium2 Bass kernel for nn_EncoderOnlyBlock (4-head full-dim encoder block).

Sharding: fully data-parallel, no collectives. 8 cores = (batch b, seq-half).
Each core computes its 1024 query tokens end-to-end for all 4 heads.

Algebraic folds (host, fp32):
  scores[i,j] = (x_i Wq + bq)·(x_j Wk + bk) / 32
              = (x_i G + w~)·x_j / 32  + terms constant over j (softmax-invariant)
    with G = Wq Wk^T, w~ = Wk bq.  K projection eliminated entirely.
  head_h @ W1_h = A_h @ (x F_h),  F_h = Wv_h W1_h;  bv/b1 fold into
    cvec = b1 + sum_h bv_h W1_h which is folded into xres on host.
  Softmax normalization deferred: proj_unnorm = sum_j exp(s) V', scaled by
    rec = 1/rowsum at PSUM->SBUF accumulation ([P,1] per-partition scalar).

fp8 (e4m3, DoubleRow perf mode = 2 K-tiles per pass) for all attention
matmuls: T' = x G'8, scores = T'8 @ x8, V' = x8 @ F'8, proj = expS8 @ V'8.
Host scales G'=32G, F'=64F keep fp8 operands in the normal range; the 32
folds into the exp scale (1/1024), the 64 into the u1 residual STT (1/64).
W2/LN tail stays bf16/f32 exactly as the bf16 baseline.

Per-core math:
  x_perm = [own-half tokens; other-half tokens]            (host permute)
  V'_h  = x_perm @ F'_h                     [sj, d]  fp8 out
  T'_h  = G'_h^T @ x_perm^T[:, :1024] + w~' [e, si]  fp8 out
  S     = T'^T x^T / 1024; E = exp(S) (fp8), r = rowsum(E) (f32 accum)
  proj += (E_h @ V'_h) * (1/r_h)            [si, d]  f32 accum in SBUF
  u1    = xres' + proj/64   (xres' = x_own + cvec, host)
  LN1 -> y; u2 = y + y@W2 + b2; LN2 -> out   (bf16 matmul, f32 LN)
"""

import numpy as np
import ml_dtypes

BF = ml_dtypes.bfloat16
F8 = ml_dtypes.float8_e4m3
P = 128
D = 1024
S = 2048
SI = 1024
H = 4
ET = D // P       # 8 e/d/f 128-blocks
SJT = S // P      # 16 sj 128-blocks
SIT = SI // P     # 8 si 128-blocks
EPS = 1e-5

_CACHE = {}


def _emit(nc, tc, A, trivial_gbe):
    """Emit the per-core program. A: dict name -> dram AP."""
    from contextlib import ExitStack

    import concourse.bass as bass
    import concourse.mybir as mybir
    from concourse.masks import make_identity

    f32 = mybir.dt.float32
    bf16 = mybir.dt.bfloat16
    f8 = mybir.dt.float8e4
    Act = mybir.ActivationFunctionType
    Alu = mybir.AluOpType
    DR = mybir.MatmulPerfMode.DoubleRow

    with ExitStack() as ctx:
        consts = ctx.enter_context(tc.tile_pool(name="consts", bufs=1))
        psA = ctx.enter_context(tc.tile_pool(name="psA", bufs=3, space="PSUM"))
        psB = ctx.enter_context(tc.tile_pool(name="psB", bufs=2, space="PSUM"))

        identb = consts.tile([P, P], bf16, tag="identb")
        make_identity(nc, identb[:])
        wtr_sb = consts.tile([P, H * ET], f32, tag="wtr")
        nc.sync.dma_start(out=wtr_sb[:], in_=A["wtr"][:])
        buv_sb = consts.tile([1, D], bf16, tag="buv")
        nc.sync.dma_start(out=buv_sb[:], in_=A["buv"][:])
        ones_sb = consts.tile([1, P], bf16, tag="ones")
        nc.vector.memset(ones_sb[:], 1.0)
        ones8 = consts.tile([P, 2, P], f8, tag="ones8")
        nc.vector.memset(ones8[:], 1.0)
        eps_sb = consts.tile([P, 1], f32, tag="eps")
        nc.vector.memset(eps_sb[:], EPS)
        nl16_sb = consts.tile([P, 1], f32, tag="nl16")
        nc.vector.memset(nl16_sb[:], -2.772588722239781)

        head_ctx = ExitStack()
        # LN1 pools live in the outer ctx: stage A (u1 -> LN1 -> y) is emitted
        # inline as head 3's proj tiles complete, so the LN phase starts hot
        lnp = ctx.enter_context(tc.tile_pool(name="lnp", bufs=1))
        u_pool = ctx.enter_context(tc.tile_pool(name="up", bufs=2))
        sq_pool = ctx.enter_context(tc.tile_pool(name="sq", bufs=2))
        st_pool = ctx.enter_context(tc.tile_pool(name="st", bufs=8))
        y_sb = lnp.tile([P, SIT, D], bf16, tag="y")

        xpool = head_ctx.enter_context(tc.tile_pool(name="xp", bufs=1))
        f_pool = head_ctx.enter_context(tc.tile_pool(name="fp8", bufs=2))
        g_pool = head_ctx.enter_context(tc.tile_pool(name="gp8", bufs=2))
        mu_pool = head_ctx.enter_context(tc.tile_pool(name="mu", bufs=2))
        tt_pool = head_ctx.enter_context(tc.tile_pool(name="tt", bufs=2))
        e_pool = head_ctx.enter_context(tc.tile_pool(name="es", bufs=1))
        proj_pool = head_ctx.enter_context(tc.tile_pool(name="pj", bufs=1))
        red_pool = head_ctx.enter_context(tc.tile_pool(name="red", bufs=1))
        rec_pool = head_ctx.enter_context(tc.tile_pool(name="rec", bufs=2))

        xq8_sb = xpool.tile([P, ET, SI], f8, tag="xq8")
        for c2 in range(4):
            nc.sync.dma_start(
                out=xq8_sb[:, 2 * c2:2 * c2 + 2, :],
                in_=A["xq8"][2 * c2 * P:(2 * c2 + 2) * P, :].rearrange(
                    "(c p) s -> p c s", p=P),
            )
        x8_sb = xpool.tile([P, ET, S], f8, tag="x8")
        for c2 in range(4):
            nc.gpsimd.dma_start(
                out=x8_sb[:, 2 * c2:2 * c2 + 2, :],
                in_=A["x8"][2 * c2 * P:(2 * c2 + 2) * P, :].rearrange(
                    "(c p) s -> p c s", p=P),
            )
        xn8_sb = xpool.tile([P, SJT, D], f8, tag="xn8")
        for j4 in range(4):
            nc.gpsimd.dma_start(
                out=xn8_sb[:, 4 * j4:4 * j4 + 4, :],
                in_=A["xn8"][4 * j4 * P:(4 * j4 + 4) * P, :].rearrange(
                    "(j p) d -> p j d", p=P),
            )

        # proj accumulator preloaded with 64*(x_own + cvec) in f32: the
        # residual + attention bias ride along exactly; u1 = proj/64.
        # (Init DMAs are emitted inside head 0 so they don't contend with the
        # startup-critical xq8/g8/x8 loads for HBM bandwidth.)
        proj_sb = proj_pool.tile([P, SIT, D], f32, tag="proj")

        if not trivial_gbe:
            gbe_sb = lnp.tile([P, 4, D], f32, tag="gbe")
            gbe_bc = bass.AP(
                tensor=A["gbe"].tensor, offset=A["gbe"].offset,
                ap=[[0, P], A["gbe"].ap[0], A["gbe"].ap[1]],
            )
            nc.gpsimd.dma_start(out=gbe_sb[:], in_=gbe_bc)

        def ln_stats(src, rsum):
            """-> (mu, rstd) [P,1] tiles from src [P,D] + its row-sum."""
            sq = sq_pool.tile([P, D], f32, tag="sq")
            sumsq = st_pool.tile([P, 1], f32, tag="sumsq")
            nc.scalar.activation(out=sq[:], in_=src, func=Act.Square,
                                 accum_out=sumsq[:])
            mu = st_pool.tile([P, 1], f32, tag="mu")
            nc.scalar.mul(mu[:], rsum, 1.0 / D)
            # (rsum*mu - sumsq) = -D*var;  std = sqrt(-1/D * that + eps)
            nv = st_pool.tile([P, 1], f32, tag="nv")
            nc.vector.scalar_tensor_tensor(
                out=nv[:], in0=rsum, scalar=mu[:], in1=sumsq[:],
                op0=Alu.mult, op1=Alu.subtract,
            )
            rstd = st_pool.tile([P, 1], f32, tag="rstd")
            nc.scalar.activation(out=rstd[:], in_=nv[:], func=Act.Sqrt,
                                 scale=-1.0 / D, bias=eps_sb[:])
            nc.vector.reciprocal(rstd[:], rstd[:])
            return mu, rstd

        def stage_A(t):
            """u1 = proj/64 -> LN1 -> y_sb[:, t]."""
            u1 = u_pool.tile([P, D], f32, tag="u")
            rs1 = st_pool.tile([P, 1], f32, tag="rs")
            nc.scalar.activation(
                out=u1[:], in_=proj_sb[:, t, :], func=Act.Copy,
                scale=1.0 / 64.0, accum_out=rs1[:],
            )
            mu1, rstd1 = ln_stats(u1[:], rs1[:])
            yt_t = y_sb[:, t, :]
            nc.vector.tensor_scalar(
                yt_t, u1[:], scalar1=mu1[:], scalar2=rstd1[:],
                op0=Alu.subtract, op1=Alu.mult,
            )
            if not trivial_gbe:
                nc.gpsimd.tensor_mul(yt_t, yt_t, gbe_sb[:, 0, :])
                nc.gpsimd.tensor_add(yt_t, yt_t, gbe_sb[:, 1, :])

        for h in range(H):
            # ---- T'^T = G'^T @ x^T[:, :1024] + w~' : [e, si]
            dmae = nc.sync if h == 0 else nc.scalar
            f8_sb = f_pool.tile([P, ET, D], f8, tag="f8")
            for k2 in range(2):
                dmae.dma_start(
                    out=f8_sb[:, 4 * k2:4 * k2 + 4, :],
                    in_=A["f8"][h, 4 * k2 * P:(4 * k2 + 4) * P, :].rearrange(
                        "(k p) d -> p k d", p=P),
                )
            g8_sb = g_pool.tile([P, ET, ET, P], f8, tag="g8")
            for c2 in range(2):
                dmae.dma_start(
                    out=g8_sb[:, 4 * c2:4 * c2 + 4], in_=A["g8"][h, :, 4 * c2:4 * c2 + 4]
                )
            tt_sb = tt_pool.tile([P, ET, SI], f8, tag="tt")
            for c in range(ET):
                ps = psA.tile([P, D], f32, tag="psA")
                for nb in range(2):
                    for kp in range(ET // 2):
                        nc.tensor.matmul(
                            ps[:, nb * 512:(nb + 1) * 512],
                            lhsT=g8_sb[:, c, 2 * kp:2 * kp + 2, :],
                            rhs=xq8_sb[:, 2 * kp:2 * kp + 2, nb * 512:(nb + 1) * 512],
                            start=(kp == 0), stop=(kp == ET // 2 - 1),
                            perf_mode=DR,
                        )
                nc.scalar.activation(
                    out=tt_sb[:, c, :], in_=ps[:], func=Act.Identity,
                    bias=wtr_sb[:, h * ET + c:h * ET + c + 1],
                )
            # ---- scores^T per key-block: S^T[j, i] = x_j . T'_i, exp'd straight
            # to fp8 (feeds proj as lhsT with no transposes). Row sums via tiny
            # ones-DoubleRow matmuls accumulated over all 16 key blocks.
            expS_sb = e_pool.tile([P, SJT, SI], f8, tag="expS")
            for jblk in range(SJT):
                ps = psA.tile([P, SI], f32, tag="psA")
                for nb in range(2):
                    for kp in range(ET // 2):
                        nc.tensor.matmul(
                            ps[:, nb * 512:(nb + 1) * 512],
                            lhsT=x8_sb[:, 2 * kp:2 * kp + 2, jblk * P:(jblk + 1) * P],
                            rhs=tt_sb[:, 2 * kp:2 * kp + 2, nb * 512:(nb + 1) * 512],
                            start=(kp == 0), stop=(kp == ET // 2 - 1),
                            perf_mode=DR,
                        )
                # exp/16: keeps M = expS @ x inside fp8 range; the 16 cancels
                # against the rowsum in the final rec normalization
                nc.scalar.activation(
                    out=expS_sb[:, jblk, :], in_=ps[:],
                    func=Act.Exp, scale=1.0 / 1024.0, bias=nl16_sb[:],
                )
            # rowsum broadcast to all 128 partitions via all-ones lhsT
            rps = psA.tile([P, SI], f32, tag="psA")
            for jp in range(SJT // 2):
                for nb in range(2):
                    nc.tensor.matmul(
                        rps[:, nb * 512:(nb + 1) * 512],
                        lhsT=ones8[:],
                        rhs=expS_sb[:, 2 * jp:2 * jp + 2, nb * 512:(nb + 1) * 512],
                        start=(jp == 0), stop=(jp == SJT // 2 - 1),
                        perf_mode=DR,
                    )
            # 1/rowsum, regrouped to [P si-in-tile, tile] via scatter DMA
            r_sb = red_pool.tile([P, SI], f32, tag="rrow")
            nc.vector.tensor_copy(r_sb[:], rps[:])
            nc.sync.dma_start(out=A["rsc"][h], in_=r_sb[0:1, :])
            rec_sb = rec_pool.tile([P, SIT], f32, tag="rec")
            nc.sync.dma_start(
                out=rec_sb[:], in_=A["rsc"][h].rearrange("(t p) -> p t", p=P)
            )
            nc.vector.reciprocal(rec_sb[:], rec_sb[:])

            # ---- M^T = x^T @ expS^T : [d, si]  (the (A@x) reassociation —
            # contracts the 2048-key dim before touching F)
            mu8 = mu_pool.tile([P, ET, SI], f8, tag="mu8")
            for dblk in range(ET):
                ps = psA.tile([P, SI], f32, tag="psA")
                for nb in range(2):
                    for jp in range(SJT // 2):
                        nc.tensor.matmul(
                            ps[:, nb * 512:(nb + 1) * 512],
                            lhsT=xn8_sb[:, 2 * jp:2 * jp + 2,
                                        dblk * P:(dblk + 1) * P],
                            rhs=expS_sb[:, 2 * jp:2 * jp + 2,
                                        nb * 512:(nb + 1) * 512],
                            start=(jp == 0), stop=(jp == SJT // 2 - 1),
                            perf_mode=DR,
                        )
                nc.scalar.copy(mu8[:, dblk, :], ps[:])

            # ---- proj += (M @ F'_h) * rec_h
            for t in range(SIT):
                ps = psA.tile([P, D], f32, tag="psA")
                for nb in range(2):
                    for kp in range(ET // 2):
                        nc.tensor.matmul(
                            ps[:, nb * 512:(nb + 1) * 512],
                            lhsT=mu8[:, 2 * kp:2 * kp + 2, t * P:(t + 1) * P],
                            rhs=f8_sb[:, 2 * kp:2 * kp + 2, nb * 512:(nb + 1) * 512],
                            start=(kp == 0), stop=(kp == ET // 2 - 1),
                            perf_mode=DR,
                        )
                if h == 0:
                    nc.vector.tensor_scalar_mul(
                        proj_sb[:, t, :], ps[:], rec_sb[:, t:t + 1]
                    )
                    # 64*xres lands via accumulating DMA: its RAW dependency on
                    # this head-0 tile keeps the 4MB transfer out of the
                    # startup HBM window
                    nc.gpsimd.dma_start(
                        out=proj_sb[:, t, :],
                        in_=A["xres"][t * P:(t + 1) * P, :],
                        accum_op=Alu.add,
                    )
                else:
                    nc.vector.scalar_tensor_tensor(
                        out=proj_sb[:, t, :], in0=ps[:], scalar=rec_sb[:, t:t + 1],
                        in1=proj_sb[:, t, :], op0=Alu.mult, op1=Alu.add,
                    )
                if h == H - 1:
                    stage_A(t)

        head_ctx.close()

        # ================= yT -> FFN2 -> LN2 (y already computed inline) ======
        with ExitStack() as lctx:
            yt_pool = lctx.enter_context(tc.tile_pool(name="yt", bufs=3))
            w2_pool = lctx.enter_context(tc.tile_pool(name="w2", bufs=1))
            ot_pool = lctx.enter_context(tc.tile_pool(name="ot", bufs=3))

            w28_sb = w2_pool.tile([P, ET, D], bf16, tag="w2")
            nc.sync.dma_start(
                out=w28_sb[:], in_=A["w2"][:].rearrange("(k p) d -> p k d", p=P)
            )

            yt_tiles = []
            for t in range(SIT):
                # transpose this tile's 8 f-blocks -> yT columns for its z-chain
                yt_t = y_sb[:, t, :]
                yt_tile = yt_pool.tile([P, ET, P], bf16, tag="yt", name=f"yt{t}")
                pb = psB.tile([P, 1024], bf16, tag="psB")
                for fb in range(ET):
                    nc.tensor.transpose(
                        pb[:, fb * P:(fb + 1) * P], yt_t[:, fb * P:(fb + 1) * P],
                        identb[:],
                    )
                nc.vector.tensor_copy(
                    yt_tile[:], pb[:].rearrange("p (f c) -> p f c", c=P)
                )
                yt_tiles.append(yt_tile)

            # Stage B (all tiles): z-chain -> u2 -> LN2 -> out
            for t in range(SIT):
                ps = psA.tile([P, 1024], f32, tag="psA")
                for nb in range(2):
                    for kc in range(ET):
                        nc.tensor.matmul(
                            ps[:, nb * 512:(nb + 1) * 512],
                            lhsT=yt_tiles[t][:, kc, :],
                            rhs=w28_sb[:, kc, nb * 512:(nb + 1) * 512],
                            start=(kc == 0), stop=False,
                        )
                    nc.tensor.matmul(
                        ps[:, nb * 512:(nb + 1) * 512],
                        lhsT=ones_sb[:, :],
                        rhs=buv_sb[:, nb * 512:(nb + 1) * 512],
                        start=False, stop=True,
                    )
                u2 = u_pool.tile([P, D], f32, tag="u")
                rs2 = st_pool.tile([P, 1], f32, tag="rs")
                nc.vector.scalar_tensor_tensor(
                    out=u2[:], in0=ps[:], scalar=1.0,
                    in1=y_sb[:, t, :], op0=Alu.mult, op1=Alu.add,
                    accum_out=rs2[:],
                )
                mu2, rstd2 = ln_stats(u2[:], rs2[:])
                ot = ot_pool.tile([P, D], f32, tag="ot")
                nc.vector.tensor_scalar(
                    ot[:], u2[:], scalar1=mu2[:], scalar2=rstd2[:],
                    op0=Alu.subtract, op1=Alu.mult,
                )
                if not trivial_gbe:
                    nc.gpsimd.tensor_mul(ot[:], ot[:], gbe_sb[:, 2, :])
                    nc.gpsimd.tensor_add(ot[:], ot[:], gbe_sb[:, 3, :])
                eng = nc.sync if t % 2 == 0 else nc.gpsimd
                eng.dma_start(out=A["out"][t * P:(t + 1) * P, :], in_=ot[:])


def _build(trivial_gbe):
    import concourse.bass as bass
    import concourse.mybir as mybir
    import concourse.tile as tile
    from concourse import bacc

    f32 = mybir.dt.float32
    bf16 = mybir.dt.bfloat16
    f8 = mybir.dt.float8e4

    nc = bacc.Bacc("TRN2", target_bir_lowering=False, debug=False, num_devices=8)
    A = {}

    def din(name, shape, dt):
        A[name] = nc.dram_tensor(name, shape, dt, kind="ExternalInput").ap()

    din("x8", [D, S], f8)
    din("xn8", [S, D], f8)
    din("xq8", [D, SI], f8)
    din("xres", [SI, D], f32)
    din("g8", [H, P, ET, ET, P], f8)
    din("f8", [H, D, D], f8)
    din("wtr", [P, H * ET], f32)
    din("w2", [D, D], bf16)
    din("buv", [1, D], bf16)
    if not trivial_gbe:
        din("gbe", [4, D], f32)
    A["rsc"] = nc.dram_tensor("rsc", [H, SI], f32, kind="Internal").ap()
    A["out"] = nc.dram_tensor("out", [SI, D], f32, kind="ExternalOutput").ap()

    with tile.TileContext(nc) as tc:
        _emit(nc, tc, A, trivial_gbe)
    nc.compile()
    return nc


def _get_nc(trivial_gbe=True):
    key = ("nc", trivial_gbe)
    if key not in _CACHE:
        _CACHE[key] = _build(trivial_gbe)
    return _CACHE[key]


def _prep_inputs(inputs):
    x = np.ascontiguousarray(inputs["embedding_matrix"], dtype=np.float32)
    Wq = np.asarray(inputs["Wq"], np.float32)
    bq = np.asarray(inputs["bq"], np.float32)
    Wv = np.asarray(inputs["Wv"], np.float32)
    bv = np.asarray(inputs["bv"], np.float32)
    Wk = np.asarray(inputs["Wk"], np.float32)
    W1 = np.asarray(inputs["W1"], np.float32)
    b1 = np.asarray(inputs["b1"], np.float32)
    W2 = np.asarray(inputs["W2"], np.float32)
    b2 = np.asarray(inputs["b2"], np.float32)
    g1 = np.asarray(inputs["g1"], np.float32)
    be1 = np.asarray(inputs["be1"], np.float32)
    g2 = np.asarray(inputs["g2"], np.float32)
    be2 = np.asarray(inputs["be2"], np.float32)

    trivial = (
        np.array_equal(g1, np.ones(D, np.float32))
        and np.array_equal(g2, np.ones(D, np.float32))
        and np.array_equal(be1, np.zeros(D, np.float32))
        and np.array_equal(be2, np.zeros(D, np.float32))
    )

    # host folds
    G = np.stack([32.0 * (Wq[h] @ Wk[h].T) for h in range(H)])        # [H, D, D]
    F = np.stack([64.0 * (Wv[h] @ W1[h * D:(h + 1) * D]) for h in range(H)])
    wt = np.stack([32.0 * (Wk[h] @ bq[h]) for h in range(H)])          # [H, D]
    cvec = b1 + sum(bv[h] @ W1[h * D:(h + 1) * D] for h in range(H))   # [D]

    def pack_w(W, dtyp):  # [H, D, D] -> [H, P(in-blk), c, kc, P] lhsT blocks
        return np.ascontiguousarray(
            W.reshape(H, ET, P, ET, P).transpose(0, 2, 3, 1, 4).astype(dtyp)
        )

    g8 = pack_w(G, F8)
    f8 = np.ascontiguousarray(F.astype(F8))
    wtr = np.ascontiguousarray(wt.reshape(H, ET, P).transpose(2, 0, 1).reshape(P, H * ET))
    w2b = np.ascontiguousarray(W2.astype(BF))
    buv = np.ascontiguousarray(b2.reshape(1, D).astype(BF))

    shared = {"g8": g8, "f8": f8, "wtr": wtr, "w2": w2b, "buv": buv}
    if not trivial:
        shared["gbe"] = np.ascontiguousarray(np.stack([g1, be1, g2, be2]))
    in_maps = []
    xT8 = [np.ascontiguousarray(x[b].T.astype(F8)) for b in range(4)]
    xN8 = [np.ascontiguousarray(x[b].astype(F8)) for b in range(4)]
    for core in range(8):
        b, half = core // 2, core % 2
        own = x[b, half * SI:(half + 1) * SI]
        m = dict(shared)
        m["x8"] = xT8[b]
        m["xn8"] = xN8[b]
        m["xq8"] = np.ascontiguousarray(xT8[b][:, half * SI:(half + 1) * SI])
        m["xres"] = np.ascontiguousarray(64.0 * (own + cvec[None, :]))
        in_maps.append(m)
    return trivial, in_maps


def kernel(**inputs):
    from concourse.bass_utils import run_bass_kernel_spmd

    trivial, in_maps = _prep_inputs(inputs)
    nc = _get_nc(trivial)
    res = run_bass_kernel_spmd(nc, in_maps, core_ids=list(range(8)))
    out = np.empty((4, S, D), np.float32)
    for core in range(8):
        b, half = core // 2, core % 2
        out[b, half * SI:(half + 1) * SI] = res.results[core]["out"]
    return out


# revision 103
# speedup vs baseline: 1.0228x; 1.0045x over previous
"""Trainium2 Bass kernel for nn_EncoderOnlyBlock (4-head full-dim encoder block).

Sharding: fully data-parallel, no collectives. 8 cores = (batch b, seq-half).
Each core computes its 1024 query tokens end-to-end for all 4 heads.

Algebraic folds (host, fp32):
  scores[i,j] = (x_i Wq + bq)·(x_j Wk + bk) / 32
              = (x_i G + w~)·x_j / 32  + terms constant over j (softmax-invariant)
    with G = Wq Wk^T, w~ = Wk bq.  K projection eliminated entirely.
  head_h @ W1_h = A_h @ (x F_h),  F_h = Wv_h W1_h;  bv/b1 fold into
    cvec = b1 + sum_h bv_h W1_h which is folded into xres on host.
  Softmax normalization deferred: proj_unnorm = sum_j exp(s) V', scaled by
    rec = 1/rowsum at PSUM->SBUF accumulation ([P,1] per-partition scalar).

fp8 (e4m3, DoubleRow perf mode = 2 K-tiles per pass) for all attention
matmuls: T' = x G'8, scores = T'8 @ x8, V' = x8 @ F'8, proj = expS8 @ V'8.
Host scales G'=32G, F'=64F keep fp8 operands in the normal range; the 32
folds into the exp scale (1/1024), the 64 into the u1 residual STT (1/64).
W2/LN tail stays bf16/f32 exactly as the bf16 baseline.

Per-core math:
  x_perm = [own-half tokens; other-half tokens]            (host permute)
  V'_h  = x_perm @ F'_h                     [sj, d]  fp8 out
  T'_h  = G'_h^T @ x_perm^T[:, :1024] + w~' [e, si]  fp8 out
  S     = T'^T x^T / 1024; E = exp(S) (fp8), r = rowsum(E) (f32 accum)
  proj += (E_h @ V'_h) * (1/r_h)            [si, d]  f32 accum in SBUF
  u1    = xres' + proj/64   (xres' = x_own + cvec, host)
  LN1 -> y; u2 = y + y@W2 + b2; LN2 -> out   (bf16 matmul, f32 LN)
"""

import numpy as np
import ml_dtypes

BF = ml_dtypes.bfloat16
F8 = ml_dtypes.float8_e4m3
P = 128
D = 1024
S = 2048
SI = 1024
H = 4
ET = D // P       # 8 e/d/f 128-blocks
SJT = S // P      # 16 sj 128-blocks
SIT = SI // P     # 8 si 128-blocks
EPS = 1e-5

_CACHE = {}


def _emit(nc, tc, A, trivial_gbe):
    """Emit the per-core program. A: dict name -> dram AP."""
    from contextlib import ExitStack

    import concourse.bass as bass
    import concourse.mybir as mybir
    from concourse.masks import make_identity

    f32 = mybir.dt.float32
    bf16 = mybir.dt.bfloat16
    f8 = mybir.dt.float8e4
    Act = mybir.ActivationFunctionType
    Alu = mybir.AluOpType
    DR = mybir.MatmulPerfMode.DoubleRow

    with ExitStack() as ctx:
        consts = ctx.enter_context(tc.tile_pool(name="consts", bufs=1))
        psA = ctx.enter_context(tc.tile_pool(name="psA", bufs=3, space="PSUM"))
        psB = ctx.enter_context(tc.tile_pool(name="psB", bufs=2, space="PSUM"))

        identb = consts.tile([P, P], bf16, tag="identb")
        make_identity(nc, identb[:])
        wtr_sb = consts.tile([P, H * ET], f32, tag="wtr")
        nc.sync.dma_start(out=wtr_sb[:], in_=A["wtr"][:])
        buv_sb = consts.tile([1, D], bf16, tag="buv")
        nc.sync.dma_start(out=buv_sb[:], in_=A["buv"][:])
        ones_sb = consts.tile([1, P], bf16, tag="ones")
        nc.vector.memset(ones_sb[:], 1.0)
        ones8 = consts.tile([P, 2, P], f8, tag="ones8")
        nc.vector.memset(ones8[:], 1.0)
        eps_sb = consts.tile([P, 1], f32, tag="eps")
        nc.vector.memset(eps_sb[:], EPS)
        nl16_sb = consts.tile([P, 1], f32, tag="nl16")
        nc.vector.memset(nl16_sb[:], -2.772588722239781)

        head_ctx = ExitStack()
        # LN1 pools live in the outer ctx: stage A (u1 -> LN1 -> y) is emitted
        # inline as head 3's proj tiles complete, so the LN phase starts hot
        lnp = ctx.enter_context(tc.tile_pool(name="lnp", bufs=1))
        u_pool = ctx.enter_context(tc.tile_pool(name="up", bufs=2))
        sq_pool = ctx.enter_context(tc.tile_pool(name="sq", bufs=2))
        st_pool = ctx.enter_context(tc.tile_pool(name="st", bufs=8))
        y_sb = lnp.tile([P, SIT, D], bf16, tag="y")

        xpool = head_ctx.enter_context(tc.tile_pool(name="xp", bufs=1))
        f_pool = head_ctx.enter_context(tc.tile_pool(name="fp8", bufs=2))
        g_pool = head_ctx.enter_context(tc.tile_pool(name="gp8", bufs=2))
        mu_pool = head_ctx.enter_context(tc.tile_pool(name="mu", bufs=2))
        tt_pool = head_ctx.enter_context(tc.tile_pool(name="tt", bufs=2))
        e_pool = head_ctx.enter_context(tc.tile_pool(name="es", bufs=1))
        proj_pool = head_ctx.enter_context(tc.tile_pool(name="pj", bufs=1))
        red_pool = head_ctx.enter_context(tc.tile_pool(name="red", bufs=1))
        rec_pool = head_ctx.enter_context(tc.tile_pool(name="rec", bufs=2))

        # x columns/rows in per-core [own-half, other-half] order; own first so
        # TT + the first scores chains can start after ~1.5MB of DMA
        x8_sb = xpool.tile([P, ET, S], f8, tag="x8")
        for hs in range(2):
            for c4 in range(2):
                nc.sync.dma_start(
                    out=x8_sb[:, 4 * c4:4 * c4 + 4, hs * SI:(hs + 1) * SI],
                    in_=A["x8"][4 * c4 * P:(4 * c4 + 4) * P,
                                hs * SI:(hs + 1) * SI].rearrange(
                        "(c p) s -> p c s", p=P),
                )
        xq8_sb = x8_sb
        xn8_sb = xpool.tile([P, SJT, D], f8, tag="xn8")
        for j4 in range(4):
            nc.gpsimd.dma_start(
                out=xn8_sb[:, 4 * j4:4 * j4 + 4, :],
                in_=A["xn8"][4 * j4 * P:(4 * j4 + 4) * P, :].rearrange(
                    "(j p) d -> p j d", p=P),
            )

        # proj accumulator preloaded with 64*(x_own + cvec) in f32: the
        # residual + attention bias ride along exactly; u1 = proj/64.
        # (Init DMAs are emitted inside head 0 so they don't contend with the
        # startup-critical xq8/g8/x8 loads for HBM bandwidth.)
        proj_sb = proj_pool.tile([P, SIT, D], f32, tag="proj")

        if not trivial_gbe:
            gbe_sb = lnp.tile([P, 4, D], f32, tag="gbe")
            gbe_bc = bass.AP(
                tensor=A["gbe"].tensor, offset=A["gbe"].offset,
                ap=[[0, P], A["gbe"].ap[0], A["gbe"].ap[1]],
            )
            nc.gpsimd.dma_start(out=gbe_sb[:], in_=gbe_bc)

        def ln_stats(src, rsum):
            """-> (mu, rstd) [P,1] tiles from src [P,D] + its row-sum."""
            sq = sq_pool.tile([P, D], f32, tag="sq")
            sumsq = st_pool.tile([P, 1], f32, tag="sumsq")
            nc.scalar.activation(out=sq[:], in_=src, func=Act.Square,
                                 accum_out=sumsq[:])
            mu = st_pool.tile([P, 1], f32, tag="mu")
            nc.scalar.mul(mu[:], rsum, 1.0 / D)
            # (rsum*mu - sumsq) = -D*var;  std = sqrt(-1/D * that + eps)
            nv = st_pool.tile([P, 1], f32, tag="nv")
            nc.vector.scalar_tensor_tensor(
                out=nv[:], in0=rsum, scalar=mu[:], in1=sumsq[:],
                op0=Alu.mult, op1=Alu.subtract,
            )
            rstd = st_pool.tile([P, 1], f32, tag="rstd")
            nc.scalar.activation(out=rstd[:], in_=nv[:], func=Act.Sqrt,
                                 scale=-1.0 / D, bias=eps_sb[:])
            nc.vector.reciprocal(rstd[:], rstd[:])
            return mu, rstd

        def stage_A(t):
            """u1 = proj/64 -> LN1 -> y_sb[:, t]."""
            u1 = u_pool.tile([P, D], f32, tag="u")
            rs1 = st_pool.tile([P, 1], f32, tag="rs")
            nc.scalar.activation(
                out=u1[:], in_=proj_sb[:, t, :], func=Act.Copy,
                scale=1.0 / 64.0, accum_out=rs1[:],
            )
            mu1, rstd1 = ln_stats(u1[:], rs1[:])
            yt_t = y_sb[:, t, :]
            nc.vector.tensor_scalar(
                yt_t, u1[:], scalar1=mu1[:], scalar2=rstd1[:],
                op0=Alu.subtract, op1=Alu.mult,
            )
            if not trivial_gbe:
                nc.gpsimd.tensor_mul(yt_t, yt_t, gbe_sb[:, 0, :])
                nc.gpsimd.tensor_add(yt_t, yt_t, gbe_sb[:, 1, :])

        for h in range(H):
            # ---- T'^T = G'^T @ x^T[:, :1024] + w~' : [e, si]
            dmae = nc.sync if h == 0 else nc.scalar
            f8_sb = f_pool.tile([P, ET, D], f8, tag="f8")
            g8_sb = g_pool.tile([P, ET, ET, P], f8, tag="g8")
            if h > 0:
                # tiny writes on the vector queue (sequenced behind head h-1's
                # proj STTs) gate these loads out of the startup HBM window
                nc.vector.memset(f8_sb[:, 0, 0:8], 0.0)
                nc.vector.memset(g8_sb[:, 0, 0, 0:8], 0.0)
            for k2 in range(2):
                dmae.dma_start(
                    out=f8_sb[:, 4 * k2:4 * k2 + 4, :],
                    in_=A["f8"][h, 4 * k2 * P:(4 * k2 + 4) * P, :].rearrange(
                        "(k p) d -> p k d", p=P),
                )
            for c2 in range(2):
                dmae.dma_start(
                    out=g8_sb[:, 4 * c2:4 * c2 + 4], in_=A["g8"][h, :, 4 * c2:4 * c2 + 4]
                )
            tt_sb = tt_pool.tile([P, ET, SI], f8, tag="tt")
            for c in range(ET):
                ps = psA.tile([P, D], f32, tag="psA")
                for nb in range(2):
                    for kp in range(ET // 2):
                        nc.tensor.matmul(
                            ps[:, nb * 512:(nb + 1) * 512],
                            lhsT=g8_sb[:, c, 2 * kp:2 * kp + 2, :],
                            rhs=xq8_sb[:, 2 * kp:2 * kp + 2, nb * 512:(nb + 1) * 512],
                            start=(kp == 0), stop=(kp == ET // 2 - 1),
                            perf_mode=DR,
                        )
                nc.scalar.activation(
                    out=tt_sb[:, c, :], in_=ps[:], func=Act.Identity,
                    bias=wtr_sb[:, h * ET + c:h * ET + c + 1],
                )
            # ---- scores^T per key-block: S^T[j, i] = x_j . T'_i, exp'd straight
            # to fp8 (feeds proj as lhsT with no transposes). Row sums via tiny
            # ones-DoubleRow matmuls accumulated over all 16 key blocks.
            expS_sb = e_pool.tile([P, SJT, SI], f8, tag="expS")
            for jblk in range(SJT):
                ps = psA.tile([P, SI], f32, tag="psA")
                for nb in range(2):
                    for kp in range(ET // 2):
                        nc.tensor.matmul(
                            ps[:, nb * 512:(nb + 1) * 512],
                            lhsT=x8_sb[:, 2 * kp:2 * kp + 2, jblk * P:(jblk + 1) * P],
                            rhs=tt_sb[:, 2 * kp:2 * kp + 2, nb * 512:(nb + 1) * 512],
                            start=(kp == 0), stop=(kp == ET // 2 - 1),
                            perf_mode=DR,
                        )
                # exp/16: keeps M = expS @ x inside fp8 range; the 16 cancels
                # against the rowsum in the final rec normalization
                nc.scalar.activation(
                    out=expS_sb[:, jblk, :], in_=ps[:],
                    func=Act.Exp, scale=1.0 / 1024.0, bias=nl16_sb[:],
                )
            # rowsum broadcast to all 128 partitions via all-ones lhsT
            rps = psA.tile([P, SI], f32, tag="psA")
            for jp in range(SJT // 2):
                for nb in range(2):
                    nc.tensor.matmul(
                        rps[:, nb * 512:(nb + 1) * 512],
                        lhsT=ones8[:],
                        rhs=expS_sb[:, 2 * jp:2 * jp + 2, nb * 512:(nb + 1) * 512],
                        start=(jp == 0), stop=(jp == SJT // 2 - 1),
                        perf_mode=DR,
                    )
            # 1/rowsum, regrouped to [P si-in-tile, tile] via scatter DMA
            r_sb = red_pool.tile([P, SI], f32, tag="rrow")
            nc.vector.tensor_copy(r_sb[:], rps[:])
            nc.sync.dma_start(out=A["rsc"][h], in_=r_sb[0:1, :])
            rec_sb = rec_pool.tile([P, SIT], f32, tag="rec")
            nc.sync.dma_start(
                out=rec_sb[:], in_=A["rsc"][h].rearrange("(t p) -> p t", p=P)
            )
            nc.vector.reciprocal(rec_sb[:], rec_sb[:])

            # ---- M^T = x^T @ expS^T : [d, si]  (the (A@x) reassociation —
            # contracts the 2048-key dim before touching F)
            mu8 = mu_pool.tile([P, ET, SI], f8, tag="mu8")
            for dblk in range(ET):
                ps = psA.tile([P, SI], f32, tag="psA")
                for nb in range(2):
                    for jp in range(SJT // 2):
                        nc.tensor.matmul(
                            ps[:, nb * 512:(nb + 1) * 512],
                            lhsT=xn8_sb[:, 2 * jp:2 * jp + 2,
                                        dblk * P:(dblk + 1) * P],
                            rhs=expS_sb[:, 2 * jp:2 * jp + 2,
                                        nb * 512:(nb + 1) * 512],
                            start=(jp == 0), stop=(jp == SJT // 2 - 1),
                            perf_mode=DR,
                        )
                nc.scalar.copy(mu8[:, dblk, :], ps[:])

            # ---- proj += (M @ F'_h) * rec_h
            for t in range(SIT):
                ps = psA.tile([P, D], f32, tag="psA")
                for nb in range(2):
                    for kp in range(ET // 2):
                        nc.tensor.matmul(
                            ps[:, nb * 512:(nb + 1) * 512],
                            lhsT=mu8[:, 2 * kp:2 * kp + 2, t * P:(t + 1) * P],
                            rhs=f8_sb[:, 2 * kp:2 * kp + 2, nb * 512:(nb + 1) * 512],
                            start=(kp == 0), stop=(kp == ET // 2 - 1),
                            perf_mode=DR,
                        )
                if h == 0:
                    nc.vector.tensor_scalar_mul(
                        proj_sb[:, t, :], ps[:], rec_sb[:, t:t + 1]
                    )
                    # 64*xres lands via accumulating DMA: its RAW dependency on
                    # this head-0 tile keeps the 4MB transfer out of the
                    # startup HBM window
                    nc.gpsimd.dma_start(
                        out=proj_sb[:, t, :],
                        in_=A["xres"][t * P:(t + 1) * P, :],
                        accum_op=Alu.add,
                    )
                else:
                    nc.vector.scalar_tensor_tensor(
                        out=proj_sb[:, t, :], in0=ps[:], scalar=rec_sb[:, t:t + 1],
                        in1=proj_sb[:, t, :], op0=Alu.mult, op1=Alu.add,
                    )
                if h == H - 1:
                    stage_A(t)

        head_ctx.close()

        # ================= yT -> FFN2 -> LN2 (y already computed inline) ======
        with ExitStack() as lctx:
            yt_pool = lctx.enter_context(tc.tile_pool(name="yt", bufs=3))
            w2_pool = lctx.enter_context(tc.tile_pool(name="w2", bufs=1))
            ot_pool = lctx.enter_context(tc.tile_pool(name="ot", bufs=3))

            w28_sb = w2_pool.tile([P, ET, D], bf16, tag="w2")
            nc.sync.dma_start(
                out=w28_sb[:], in_=A["w2"][:].rearrange("(k p) d -> p k d", p=P)
            )

            yt_tiles = []
            for t in range(SIT):
                # transpose this tile's 8 f-blocks -> yT columns for its z-chain
                yt_t = y_sb[:, t, :]
                yt_tile = yt_pool.tile([P, ET, P], bf16, tag="yt", name=f"yt{t}")
                pb = psB.tile([P, 1024], bf16, tag="psB")
                for fb in range(ET):
                    nc.tensor.transpose(
                        pb[:, fb * P:(fb + 1) * P], yt_t[:, fb * P:(fb + 1) * P],
                        identb[:],
                    )
                nc.vector.tensor_copy(
                    yt_tile[:], pb[:].rearrange("p (f c) -> p f c", c=P)
                )
                yt_tiles.append(yt_tile)

            # Stage B (all tiles): z-chain -> u2 -> LN2 -> out
            for t in range(SIT):
                ps = psA.tile([P, 1024], f32, tag="psA")
                for nb in range(2):
                    for kc in range(ET):
                        nc.tensor.matmul(
                            ps[:, nb * 512:(nb + 1) * 512],
                            lhsT=yt_tiles[t][:, kc, :],
                            rhs=w28_sb[:, kc, nb * 512:(nb + 1) * 512],
                            start=(kc == 0), stop=False,
                        )
                    nc.tensor.matmul(
                        ps[:, nb * 512:(nb + 1) * 512],
                        lhsT=ones_sb[:, :],
                        rhs=buv_sb[:, nb * 512:(nb + 1) * 512],
                        start=False, stop=True,
                    )
                u2 = u_pool.tile([P, D], f32, tag="u")
                rs2 = st_pool.tile([P, 1], f32, tag="rs")
                nc.vector.scalar_tensor_tensor(
                    out=u2[:], in0=ps[:], scalar=1.0,
                    in1=y_sb[:, t, :], op0=Alu.mult, op1=Alu.add,
                    accum_out=rs2[:],
                )
                mu2, rstd2 = ln_stats(u2[:], rs2[:])
                ot = ot_pool.tile([P, D], f32, tag="ot")
                nc.vector.tensor_scalar(
                    ot[:], u2[:], scalar1=mu2[:], scalar2=rstd2[:],
                    op0=Alu.subtract, op1=Alu.mult,
                )
                if not trivial_gbe:
                    nc.gpsimd.tensor_mul(ot[:], ot[:], gbe_sb[:, 2, :])
                    nc.gpsimd.tensor_add(ot[:], ot[:], gbe_sb[:, 3, :])
                eng = nc.sync if t % 2 == 0 else nc.gpsimd
                eng.dma_start(out=A["out"][t * P:(t + 1) * P, :], in_=ot[:])


def _build(trivial_gbe):
    import concourse.bass as bass
    import concourse.mybir as mybir
    import concourse.tile as tile
    from concourse import bacc

    f32 = mybir.dt.float32
    bf16 = mybir.dt.bfloat16
    f8 = mybir.dt.float8e4

    nc = bacc.Bacc("TRN2", target_bir_lowering=False, debug=False, num_devices=8)
    A = {}

    def din(name, shape, dt):
        A[name] = nc.dram_tensor(name, shape, dt, kind="ExternalInput").ap()

    din("x8", [D, S], f8)
    din("xn8", [S, D], f8)
    din("xres", [SI, D], f32)
    din("g8", [H, P, ET, ET, P], f8)
    din("f8", [H, D, D], f8)
    din("wtr", [P, H * ET], f32)
    din("w2", [D, D], bf16)
    din("buv", [1, D], bf16)
    if not trivial_gbe:
        din("gbe", [4, D], f32)
    A["rsc"] = nc.dram_tensor("rsc", [H, SI], f32, kind="Internal").ap()
    A["out"] = nc.dram_tensor("out", [SI, D], f32, kind="ExternalOutput").ap()

    with tile.TileContext(nc) as tc:
        _emit(nc, tc, A, trivial_gbe)
    nc.compile()
    return nc


def _get_nc(trivial_gbe=True):
    key = ("nc", trivial_gbe)
    if key not in _CACHE:
        _CACHE[key] = _build(trivial_gbe)
    return _CACHE[key]


def _prep_inputs(inputs):
    x = np.ascontiguousarray(inputs["embedding_matrix"], dtype=np.float32)
    Wq = np.asarray(inputs["Wq"], np.float32)
    bq = np.asarray(inputs["bq"], np.float32)
    Wv = np.asarray(inputs["Wv"], np.float32)
    bv = np.asarray(inputs["bv"], np.float32)
    Wk = np.asarray(inputs["Wk"], np.float32)
    W1 = np.asarray(inputs["W1"], np.float32)
    b1 = np.asarray(inputs["b1"], np.float32)
    W2 = np.asarray(inputs["W2"], np.float32)
    b2 = np.asarray(inputs["b2"], np.float32)
    g1 = np.asarray(inputs["g1"], np.float32)
    be1 = np.asarray(inputs["be1"], np.float32)
    g2 = np.asarray(inputs["g2"], np.float32)
    be2 = np.asarray(inputs["be2"], np.float32)

    trivial = (
        np.array_equal(g1, np.ones(D, np.float32))
        and np.array_equal(g2, np.ones(D, np.float32))
        and np.array_equal(be1, np.zeros(D, np.float32))
        and np.array_equal(be2, np.zeros(D, np.float32))
    )

    # host folds
    G = np.stack([32.0 * (Wq[h] @ Wk[h].T) for h in range(H)])        # [H, D, D]
    F = np.stack([64.0 * (Wv[h] @ W1[h * D:(h + 1) * D]) for h in range(H)])
    wt = np.stack([32.0 * (Wk[h] @ bq[h]) for h in range(H)])          # [H, D]
    cvec = b1 + sum(bv[h] @ W1[h * D:(h + 1) * D] for h in range(H))   # [D]

    def pack_w(W, dtyp):  # [H, D, D] -> [H, P(in-blk), c, kc, P] lhsT blocks
        return np.ascontiguousarray(
            W.reshape(H, ET, P, ET, P).transpose(0, 2, 3, 1, 4).astype(dtyp)
        )

    g8 = pack_w(G, F8)
    f8 = np.ascontiguousarray(F.astype(F8))
    wtr = np.ascontiguousarray(wt.reshape(H, ET, P).transpose(2, 0, 1).reshape(P, H * ET))
    w2b = np.ascontiguousarray(W2.astype(BF))
    buv = np.ascontiguousarray(b2.reshape(1, D).astype(BF))

    shared = {"g8": g8, "f8": f8, "wtr": wtr, "w2": w2b, "buv": buv}
    if not trivial:
        shared["gbe"] = np.ascontiguousarray(np.stack([g1, be1, g2, be2]))
    in_maps = []
    for core in range(8):
        b, half = core // 2, core % 2
        own = x[b, half * SI:(half + 1) * SI]
        other = x[b, (1 - half) * SI:(2 - half) * SI]
        xperm = np.concatenate([own, other], axis=0)
        m = dict(shared)
        xp8 = xperm.astype(F8)
        m["x8"] = np.ascontiguousarray(xp8.T)
        m["xn8"] = np.ascontiguousarray(xp8)
        m["xres"] = np.ascontiguousarray(64.0 * (own + cvec[None, :]))
        in_maps.append(m)
    return trivial, in_maps


def kernel(**inputs):
    from concourse.bass_utils import run_bass_kernel_spmd

    trivial, in_maps = _prep_inputs(inputs)
    nc = _get_nc(trivial)
    res = run_bass_kernel_spmd(nc, in_maps, core_ids=list(range(8)))
    out = np.empty((4, S, D), np.float32)
    for core in range(8):
        b, half = core // 2, core % 2
        out[b, half * SI:(half + 1) * SI] = res.results[core]["out"]
    return out


# revision 105
# speedup vs baseline: 1.0299x; 1.0070x over previous
"""Trainium2 Bass kernel for nn_EncoderOnlyBlock (4-head full-dim encoder block).

Sharding: fully data-parallel, no collectives. 8 cores = (batch b, seq-half).
Each core computes its 1024 query tokens end-to-end for all 4 heads.

Algebraic folds (host, fp32):
  scores[i,j] = (x_i Wq + bq)·(x_j Wk + bk) / 32
              = (x_i G + w~)·x_j / 32  + terms constant over j (softmax-invariant)
    with G = Wq Wk^T, w~ = Wk bq.  K projection eliminated entirely.
  head_h @ W1_h = A_h @ (x F_h),  F_h = Wv_h W1_h;  bv/b1 fold into
    cvec = b1 + sum_h bv_h W1_h which is folded into xres on host.
  Softmax normalization deferred: proj_unnorm = sum_j exp(s) V', scaled by
    rec = 1/rowsum at PSUM->SBUF accumulation ([P,1] per-partition scalar).

fp8 (e4m3, DoubleRow perf mode = 2 K-tiles per pass) for all attention
matmuls: T' = x G'8, scores = T'8 @ x8, V' = x8 @ F'8, proj = expS8 @ V'8.
Host scales G'=32G, F'=64F keep fp8 operands in the normal range; the 32
folds into the exp scale (1/1024), the 64 into the u1 residual STT (1/64).
W2/LN tail stays bf16/f32 exactly as the bf16 baseline.

Per-core math:
  x_perm = [own-half tokens; other-half tokens]            (host permute)
  V'_h  = x_perm @ F'_h                     [sj, d]  fp8 out
  T'_h  = G'_h^T @ x_perm^T[:, :1024] + w~' [e, si]  fp8 out
  S     = T'^T x^T / 1024; E = exp(S) (fp8), r = rowsum(E) (f32 accum)
  proj += (E_h @ V'_h) * (1/r_h)            [si, d]  f32 accum in SBUF
  u1    = xres' + proj/64   (xres' = x_own + cvec, host)
  LN1 -> y; u2 = y + y@W2 + b2; LN2 -> out   (bf16 matmul, f32 LN)
"""

import numpy as np
import ml_dtypes

BF = ml_dtypes.bfloat16
F8 = ml_dtypes.float8_e4m3
P = 128
D = 1024
S = 2048
SI = 1024
H = 4
ET = D // P       # 8 e/d/f 128-blocks
SJT = S // P      # 16 sj 128-blocks
SIT = SI // P     # 8 si 128-blocks
EPS = 1e-5

_CACHE = {}


def _emit(nc, tc, A, trivial_gbe):
    """Emit the per-core program. A: dict name -> dram AP."""
    from contextlib import ExitStack

    import concourse.bass as bass
    import concourse.mybir as mybir
    from concourse.masks import make_identity

    f32 = mybir.dt.float32
    bf16 = mybir.dt.bfloat16
    f8 = mybir.dt.float8e4
    Act = mybir.ActivationFunctionType
    Alu = mybir.AluOpType
    DR = mybir.MatmulPerfMode.DoubleRow

    with ExitStack() as ctx:
        consts = ctx.enter_context(tc.tile_pool(name="consts", bufs=1))
        psA = ctx.enter_context(tc.tile_pool(name="psA", bufs=3, space="PSUM"))
        psB = ctx.enter_context(tc.tile_pool(name="psB", bufs=2, space="PSUM"))

        identb = consts.tile([P, P], bf16, tag="identb")
        make_identity(nc, identb[:])
        wtr_sb = consts.tile([P, H * ET], f32, tag="wtr")
        nc.sync.dma_start(out=wtr_sb[:], in_=A["wtr"][:])
        buv_sb = consts.tile([1, D], bf16, tag="buv")
        nc.sync.dma_start(out=buv_sb[:], in_=A["buv"][:])
        ones_sb = consts.tile([1, P], bf16, tag="ones")
        nc.vector.memset(ones_sb[:], 1.0)
        ones8 = consts.tile([P, 2, P], f8, tag="ones8")
        nc.vector.memset(ones8[:], 1.0)
        eps_sb = consts.tile([P, 1], f32, tag="eps")
        nc.vector.memset(eps_sb[:], EPS)
        nl16_sb = consts.tile([P, 1], f32, tag="nl16")
        nc.vector.memset(nl16_sb[:], -2.772588722239781)

        head_ctx = ExitStack()
        # LN1 pools live in the outer ctx: stage A (u1 -> LN1 -> y) is emitted
        # inline as head 3's proj tiles complete, so the LN phase starts hot
        lnp = ctx.enter_context(tc.tile_pool(name="lnp", bufs=1))
        u_pool = ctx.enter_context(tc.tile_pool(name="up", bufs=2))
        sq_pool = ctx.enter_context(tc.tile_pool(name="sq", bufs=2))
        st_pool = ctx.enter_context(tc.tile_pool(name="st", bufs=8))
        y_sb = lnp.tile([P, SIT, D], bf16, tag="y")

        xpool = head_ctx.enter_context(tc.tile_pool(name="xp", bufs=1))
        f_pool = head_ctx.enter_context(tc.tile_pool(name="fp8", bufs=2))
        g_pool = head_ctx.enter_context(tc.tile_pool(name="gp8", bufs=2))
        mu_pool = head_ctx.enter_context(tc.tile_pool(name="mu", bufs=2))
        tt_pool = head_ctx.enter_context(tc.tile_pool(name="tt", bufs=2))
        e_pool = head_ctx.enter_context(tc.tile_pool(name="es", bufs=1))
        proj_pool = head_ctx.enter_context(tc.tile_pool(name="pj", bufs=1))
        red_pool = head_ctx.enter_context(tc.tile_pool(name="red", bufs=1))
        rec_pool = head_ctx.enter_context(tc.tile_pool(name="rec", bufs=2))

        # x columns/rows in per-core [own-half, other-half] order; own first so
        # TT + the first scores chains can start after ~1.5MB of DMA
        x8_sb = xpool.tile([P, ET, S], f8, tag="x8")
        for hs in range(2):
            for c4 in range(2):
                nc.sync.dma_start(
                    out=x8_sb[:, 4 * c4:4 * c4 + 4, hs * SI:(hs + 1) * SI],
                    in_=A["x8"][4 * c4 * P:(4 * c4 + 4) * P,
                                hs * SI:(hs + 1) * SI].rearrange(
                        "(c p) s -> p c s", p=P),
                )
        xq8_sb = x8_sb
        xn8_sb = xpool.tile([P, SJT, D], f8, tag="xn8")

        # proj accumulator preloaded with 64*(x_own + cvec) in f32: the
        # residual + attention bias ride along exactly; u1 = proj/64.
        # (Init DMAs are emitted inside head 0 so they don't contend with the
        # startup-critical xq8/g8/x8 loads for HBM bandwidth.)
        proj_sb = proj_pool.tile([P, SIT, D], f32, tag="proj")

        if not trivial_gbe:
            gbe_sb = lnp.tile([P, 4, D], f32, tag="gbe")
            gbe_bc = bass.AP(
                tensor=A["gbe"].tensor, offset=A["gbe"].offset,
                ap=[[0, P], A["gbe"].ap[0], A["gbe"].ap[1]],
            )
            nc.gpsimd.dma_start(out=gbe_sb[:], in_=gbe_bc)

        def ln_stats(src, rsum):
            """-> (mu, rstd) [P,1] tiles from src [P,D] + its row-sum."""
            sq = sq_pool.tile([P, D], f32, tag="sq")
            sumsq = st_pool.tile([P, 1], f32, tag="sumsq")
            nc.scalar.activation(out=sq[:], in_=src, func=Act.Square,
                                 accum_out=sumsq[:])
            mu = st_pool.tile([P, 1], f32, tag="mu")
            nc.scalar.mul(mu[:], rsum, 1.0 / D)
            # (rsum*mu - sumsq) = -D*var;  std = sqrt(-1/D * that + eps)
            nv = st_pool.tile([P, 1], f32, tag="nv")
            nc.vector.scalar_tensor_tensor(
                out=nv[:], in0=rsum, scalar=mu[:], in1=sumsq[:],
                op0=Alu.mult, op1=Alu.subtract,
            )
            rstd = st_pool.tile([P, 1], f32, tag="rstd")
            nc.scalar.activation(out=rstd[:], in_=nv[:], func=Act.Sqrt,
                                 scale=-1.0 / D, bias=eps_sb[:])
            nc.vector.reciprocal(rstd[:], rstd[:])
            return mu, rstd

        def stage_A(t):
            """u1 = proj/64 -> LN1 -> y_sb[:, t]."""
            u1 = u_pool.tile([P, D], f32, tag="u")
            rs1 = st_pool.tile([P, 1], f32, tag="rs")
            nc.scalar.activation(
                out=u1[:], in_=proj_sb[:, t, :], func=Act.Copy,
                scale=1.0 / 64.0, accum_out=rs1[:],
            )
            mu1, rstd1 = ln_stats(u1[:], rs1[:])
            yt_t = y_sb[:, t, :]
            nc.vector.tensor_scalar(
                yt_t, u1[:], scalar1=mu1[:], scalar2=rstd1[:],
                op0=Alu.subtract, op1=Alu.mult,
            )
            if not trivial_gbe:
                nc.gpsimd.tensor_mul(yt_t, yt_t, gbe_sb[:, 0, :])
                nc.gpsimd.tensor_add(yt_t, yt_t, gbe_sb[:, 1, :])

        for h in range(H):
            # ---- T'^T = G'^T @ x^T[:, :1024] + w~' : [e, si]
            dmae = nc.sync if h == 0 else nc.scalar
            f8_sb = f_pool.tile([P, ET, D], f8, tag="f8")
            g8_sb = g_pool.tile([P, ET, ET, P], f8, tag="g8")
            if h > 0:
                # tiny writes on the vector queue (sequenced behind head h-1's
                # proj STTs) gate these loads out of the startup HBM window
                nc.vector.memset(f8_sb[:, 0, 0:8], 0.0)
                nc.vector.memset(g8_sb[:, 0, 0, 0:8], 0.0)
            for k2 in range(2):
                dmae.dma_start(
                    out=f8_sb[:, 4 * k2:4 * k2 + 4, :],
                    in_=A["f8"][h, 4 * k2 * P:(4 * k2 + 4) * P, :].rearrange(
                        "(k p) d -> p k d", p=P),
                )
            for c2 in range(2):
                dmae.dma_start(
                    out=g8_sb[:, 4 * c2:4 * c2 + 4], in_=A["g8"][h, :, 4 * c2:4 * c2 + 4]
                )
            tt_sb = tt_pool.tile([P, ET, SI], f8, tag="tt")
            for c in range(ET):
                ps = psA.tile([P, D], f32, tag="psA")
                for nb in range(2):
                    for kp in range(ET // 2):
                        nc.tensor.matmul(
                            ps[:, nb * 512:(nb + 1) * 512],
                            lhsT=g8_sb[:, c, 2 * kp:2 * kp + 2, :],
                            rhs=xq8_sb[:, 2 * kp:2 * kp + 2, nb * 512:(nb + 1) * 512],
                            start=(kp == 0), stop=(kp == ET // 2 - 1),
                            perf_mode=DR,
                        )
                nc.scalar.activation(
                    out=tt_sb[:, c, :], in_=ps[:], func=Act.Identity,
                    bias=wtr_sb[:, h * ET + c:h * ET + c + 1],
                )
            if h == 0:
                # xn8 (M-phase input) paced behind the TT copies so it stays
                # clear of the startup-critical x8/g8/f8 HBM window
                for j4 in range(4):
                    nc.scalar.dma_start(
                        out=xn8_sb[:, 4 * j4:4 * j4 + 4, :],
                        in_=A["xn8"][4 * j4 * P:(4 * j4 + 4) * P, :].rearrange(
                            "(j p) d -> p j d", p=P),
                    )
            # ---- scores^T per key-block: S^T[j, i] = x_j . T'_i, exp'd straight
            # to fp8 (feeds proj as lhsT with no transposes). Row sums via tiny
            # ones-DoubleRow matmuls accumulated over all 16 key blocks.
            expS_sb = e_pool.tile([P, SJT, SI], f8, tag="expS")
            for jblk in range(SJT):
                ps = psA.tile([P, SI], f32, tag="psA")
                for nb in range(2):
                    for kp in range(ET // 2):
                        nc.tensor.matmul(
                            ps[:, nb * 512:(nb + 1) * 512],
                            lhsT=x8_sb[:, 2 * kp:2 * kp + 2, jblk * P:(jblk + 1) * P],
                            rhs=tt_sb[:, 2 * kp:2 * kp + 2, nb * 512:(nb + 1) * 512],
                            start=(kp == 0), stop=(kp == ET // 2 - 1),
                            perf_mode=DR,
                        )
                # exp/16: keeps M = expS @ x inside fp8 range; the 16 cancels
                # against the rowsum in the final rec normalization
                nc.scalar.activation(
                    out=expS_sb[:, jblk, :], in_=ps[:],
                    func=Act.Exp, scale=1.0 / 1024.0, bias=nl16_sb[:],
                )
            # rowsum broadcast to all 128 partitions via all-ones lhsT
            rps = psA.tile([P, SI], f32, tag="psA")
            for jp in range(SJT // 2):
                for nb in range(2):
                    nc.tensor.matmul(
                        rps[:, nb * 512:(nb + 1) * 512],
                        lhsT=ones8[:],
                        rhs=expS_sb[:, 2 * jp:2 * jp + 2, nb * 512:(nb + 1) * 512],
                        start=(jp == 0), stop=(jp == SJT // 2 - 1),
                        perf_mode=DR,
                    )
            # 1/rowsum, regrouped to [P si-in-tile, tile] via scatter DMA
            r_sb = red_pool.tile([P, SI], f32, tag="rrow")
            nc.vector.tensor_copy(r_sb[:], rps[:])
            nc.sync.dma_start(out=A["rsc"][h], in_=r_sb[0:1, :])
            rec_sb = rec_pool.tile([P, SIT], f32, tag="rec")
            nc.sync.dma_start(
                out=rec_sb[:], in_=A["rsc"][h].rearrange("(t p) -> p t", p=P)
            )
            nc.vector.reciprocal(rec_sb[:], rec_sb[:])

            # ---- M^T = x^T @ expS^T : [d, si]  (the (A@x) reassociation —
            # contracts the 2048-key dim before touching F)
            mu8 = mu_pool.tile([P, ET, SI], f8, tag="mu8")
            for dblk in range(ET):
                ps = psA.tile([P, SI], f32, tag="psA")
                for nb in range(2):
                    for jp in range(SJT // 2):
                        nc.tensor.matmul(
                            ps[:, nb * 512:(nb + 1) * 512],
                            lhsT=xn8_sb[:, 2 * jp:2 * jp + 2,
                                        dblk * P:(dblk + 1) * P],
                            rhs=expS_sb[:, 2 * jp:2 * jp + 2,
                                        nb * 512:(nb + 1) * 512],
                            start=(jp == 0), stop=(jp == SJT // 2 - 1),
                            perf_mode=DR,
                        )
                nc.scalar.copy(mu8[:, dblk, :], ps[:])

            # ---- proj += (M @ F'_h) * rec_h
            for t in range(SIT):
                ps = psA.tile([P, D], f32, tag="psA")
                for nb in range(2):
                    for kp in range(ET // 2):
                        nc.tensor.matmul(
                            ps[:, nb * 512:(nb + 1) * 512],
                            lhsT=mu8[:, 2 * kp:2 * kp + 2, t * P:(t + 1) * P],
                            rhs=f8_sb[:, 2 * kp:2 * kp + 2, nb * 512:(nb + 1) * 512],
                            start=(kp == 0), stop=(kp == ET // 2 - 1),
                            perf_mode=DR,
                        )
                if h == 0:
                    nc.vector.tensor_scalar_mul(
                        proj_sb[:, t, :], ps[:], rec_sb[:, t:t + 1]
                    )
                    # 64*xres lands via accumulating DMA: its RAW dependency on
                    # this head-0 tile keeps the 4MB transfer out of the
                    # startup HBM window
                    nc.gpsimd.dma_start(
                        out=proj_sb[:, t, :],
                        in_=A["xres"][t * P:(t + 1) * P, :],
                        accum_op=Alu.add,
                    )
                else:
                    nc.vector.scalar_tensor_tensor(
                        out=proj_sb[:, t, :], in0=ps[:], scalar=rec_sb[:, t:t + 1],
                        in1=proj_sb[:, t, :], op0=Alu.mult, op1=Alu.add,
                    )
                if h == H - 1:
                    stage_A(t)

        head_ctx.close()

        # ================= yT -> FFN2 -> LN2 (y already computed inline) ======
        with ExitStack() as lctx:
            yt_pool = lctx.enter_context(tc.tile_pool(name="yt", bufs=3))
            w2_pool = lctx.enter_context(tc.tile_pool(name="w2", bufs=1))
            ot_pool = lctx.enter_context(tc.tile_pool(name="ot", bufs=3))

            w28_sb = w2_pool.tile([P, ET, D], bf16, tag="w2")
            nc.sync.dma_start(
                out=w28_sb[:], in_=A["w2"][:].rearrange("(k p) d -> p k d", p=P)
            )

            yt_tiles = []
            for t in range(SIT):
                # transpose this tile's 8 f-blocks -> yT columns for its z-chain
                yt_t = y_sb[:, t, :]
                yt_tile = yt_pool.tile([P, ET, P], bf16, tag="yt", name=f"yt{t}")
                pb = psB.tile([P, 1024], bf16, tag="psB")
                for fb in range(ET):
                    nc.tensor.transpose(
                        pb[:, fb * P:(fb + 1) * P], yt_t[:, fb * P:(fb + 1) * P],
                        identb[:],
                    )
                nc.vector.tensor_copy(
                    yt_tile[:], pb[:].rearrange("p (f c) -> p f c", c=P)
                )
                yt_tiles.append(yt_tile)

            # Stage B (all tiles): z-chain -> u2 -> LN2 -> out
            for t in range(SIT):
                ps = psA.tile([P, 1024], f32, tag="psA")
                for nb in range(2):
                    for kc in range(ET):
                        nc.tensor.matmul(
                            ps[:, nb * 512:(nb + 1) * 512],
                            lhsT=yt_tiles[t][:, kc, :],
                            rhs=w28_sb[:, kc, nb * 512:(nb + 1) * 512],
                            start=(kc == 0), stop=False,
                        )
                    nc.tensor.matmul(
                        ps[:, nb * 512:(nb + 1) * 512],
                        lhsT=ones_sb[:, :],
                        rhs=buv_sb[:, nb * 512:(nb + 1) * 512],
                        start=False, stop=True,
                    )
                u2 = u_pool.tile([P, D], f32, tag="u")
                rs2 = st_pool.tile([P, 1], f32, tag="rs")
                nc.vector.scalar_tensor_tensor(
                    out=u2[:], in0=ps[:], scalar=1.0,
                    in1=y_sb[:, t, :], op0=Alu.mult, op1=Alu.add,
                    accum_out=rs2[:],
                )
                mu2, rstd2 = ln_stats(u2[:], rs2[:])
                ot = ot_pool.tile([P, D], f32, tag="ot")
                nc.vector.tensor_scalar(
                    ot[:], u2[:], scalar1=mu2[:], scalar2=rstd2[:],
                    op0=Alu.subtract, op1=Alu.mult,
                )
                if not trivial_gbe:
                    nc.gpsimd.tensor_mul(ot[:], ot[:], gbe_sb[:, 2, :])
                    nc.gpsimd.tensor_add(ot[:], ot[:], gbe_sb[:, 3, :])
                eng = nc.sync if t % 2 == 0 else nc.gpsimd
                eng.dma_start(out=A["out"][t * P:(t + 1) * P, :], in_=ot[:])


def _build(trivial_gbe):
    import concourse.bass as bass
    import concourse.mybir as mybir
    import concourse.tile as tile
    from concourse import bacc

    f32 = mybir.dt.float32
    bf16 = mybir.dt.bfloat16
    f8 = mybir.dt.float8e4

    nc = bacc.Bacc("TRN2", target_bir_lowering=False, debug=False, num_devices=8)
    A = {}

    def din(name, shape, dt):
        A[name] = nc.dram_tensor(name, shape, dt, kind="ExternalInput").ap()

    din("x8", [D, S], f8)
    din("xn8", [S, D], f8)
    din("xres", [SI, D], f32)
    din("g8", [H, P, ET, ET, P], f8)
    din("f8", [H, D, D], f8)
    din("wtr", [P, H * ET], f32)
    din("w2", [D, D], bf16)
    din("buv", [1, D], bf16)
    if not trivial_gbe:
        din("gbe", [4, D], f32)
    A["rsc"] = nc.dram_tensor("rsc", [H, SI], f32, kind="Internal").ap()
    A["out"] = nc.dram_tensor("out", [SI, D], f32, kind="ExternalOutput").ap()

    with tile.TileContext(nc) as tc:
        _emit(nc, tc, A, trivial_gbe)
    nc.compile()
    return nc


def _get_nc(trivial_gbe=True):
    key = ("nc", trivial_gbe)
    if key not in _CACHE:
        _CACHE[key] = _build(trivial_gbe)
    return _CACHE[key]


def _prep_inputs(inputs):
    x = np.ascontiguousarray(inputs["embedding_matrix"], dtype=np.float32)
    Wq = np.asarray(inputs["Wq"], np.float32)
    bq = np.asarray(inputs["bq"], np.float32)
    Wv = np.asarray(inputs["Wv"], np.float32)
    bv = np.asarray(inputs["bv"], np.float32)
    Wk = np.asarray(inputs["Wk"], np.float32)
    W1 = np.asarray(inputs["W1"], np.float32)
    b1 = np.asarray(inputs["b1"], np.float32)
    W2 = np.asarray(inputs["W2"], np.float32)
    b2 = np.asarray(inputs["b2"], np.float32)
    g1 = np.asarray(inputs["g1"], np.float32)
    be1 = np.asarray(inputs["be1"], np.float32)
    g2 = np.asarray(inputs["g2"], np.float32)
    be2 = np.asarray(inputs["be2"], np.float32)

    trivial = (
        np.array_equal(g1, np.ones(D, np.float32))
        and np.array_equal(g2, np.ones(D, np.float32))
        and np.array_equal(be1, np.zeros(D, np.float32))
        and np.array_equal(be2, np.zeros(D, np.float32))
    )

    # host folds
    G = np.stack([32.0 * (Wq[h] @ Wk[h].T) for h in range(H)])        # [H, D, D]
    F = np.stack([64.0 * (Wv[h] @ W1[h * D:(h + 1) * D]) for h in range(H)])
    wt = np.stack([32.0 * (Wk[h] @ bq[h]) for h in range(H)])          # [H, D]
    cvec = b1 + sum(bv[h] @ W1[h * D:(h + 1) * D] for h in range(H))   # [D]

    def pack_w(W, dtyp):  # [H, D, D] -> [H, P(in-blk), c, kc, P] lhsT blocks
        return np.ascontiguousarray(
            W.reshape(H, ET, P, ET, P).transpose(0, 2, 3, 1, 4).astype(dtyp)
        )

    g8 = pack_w(G, F8)
    f8 = np.ascontiguousarray(F.astype(F8))
    wtr = np.ascontiguousarray(wt.reshape(H, ET, P).transpose(2, 0, 1).reshape(P, H * ET))
    w2b = np.ascontiguousarray(W2.astype(BF))
    buv = np.ascontiguousarray(b2.reshape(1, D).astype(BF))

    shared = {"g8": g8, "f8": f8, "wtr": wtr, "w2": w2b, "buv": buv}
    if not trivial:
        shared["gbe"] = np.ascontiguousarray(np.stack([g1, be1, g2, be2]))
    in_maps = []
    for core in range(8):
        b, half = core // 2, core % 2
        own = x[b, half * SI:(half + 1) * SI]
        other = x[b, (1 - half) * SI:(2 - half) * SI]
        xperm = np.concatenate([own, other], axis=0)
        m = dict(shared)
        xp8 = xperm.astype(F8)
        m["x8"] = np.ascontiguousarray(xp8.T)
        m["xn8"] = np.ascontiguousarray(xp8)
        m["xres"] = np.ascontiguousarray(64.0 * (own + cvec[None, :]))
        in_maps.append(m)
    return trivial, in_maps


def kernel(**inputs):
    from concourse.bass_utils import run_bass_kernel_spmd

    trivial, in_maps = _prep_inputs(inputs)
    nc = _get_nc(trivial)
    res = run_bass_kernel_spmd(nc, in_maps, core_ids=list(range(8)))
    out = np.empty((4, S, D), np.float32)
    for core in range(8):
        b, half = core // 2, core % 2
        out[b, half * SI:(half + 1) * SI] = res.results[core]["out"]
    return out
